# revision 37
# baseline (speedup 1.0000x reference)
"""Trainium2 Bass kernel for nn_DenoisingTransformer (linear attention transformer block).

Computation (see reference):
  q,k,v = x@Wq, x@Wk, x@Wv  (split into 16 heads of 64)
  q,k = rope(q), rope(k)    (interleaved-pair rope, absolute positions)
  q,k = relu(q), relu(k)
  vk[b,h,e,d] = sum_s v_pad[b,h,s,e] * k[b,h,s,d]   (v padded with ones col, e=65)
  num[b,h,l,e] = sum_d vk[e,d] q[l,d]
  attn = num[..., :64] / (num[..., 64:] + eps)
  out = attn @ Wo

Sharding: 8 cores = (batch 4) x (sequence halves 2). Each core computes its
2048 rows end-to-end; the tiny vk state ([h,65,64] per batch) is all-reduced
between the two cores sharing a batch (replica groups [0,1],[2,3],[4,5],[6,7]).

Key layout/perf decisions:
 - x is cast to bf16 on the host; xT materializes in SBUF via 32 XBAR
   DMA-transposes straight from DRAM (no PE transposes, no casts, no psum).
 - Wq/Wk columns are de-interleaved per head host-side so rope operates on
   contiguous 32-col blocks (evens block / odds block).
 - vk is accumulated directly in 3 persistent PSUM banks across all 16 tiles
   (start at t==0, stop at t==T-1); k/v projections use 2 rotating psum banks.
 - rope is split across DVE (evens output) and Pool (odds output) so psum
   banks free fast enough for the 2-buffer rotation.
 - q projection runs after the vk collective is issued, hiding its latency;
   qT and attnT are produced by SBUF->SBUF DMA-transposes, keeping the PE
   free for the 1024 projection matmuls (the bf16 roofline of this problem).
 - phase 2b emits num(t+1) before out(t) so the attn->attnT->out chain of
   tile t hides under the matmuls of neighboring tiles.
"""

import numpy as np

import concourse.bacc as bacc
import concourse.bass as bass
import concourse.mybir as mybir
import concourse.tile as tile
from concourse.masks import make_identity

F32 = mybir.dt.float32
BF16 = mybir.dt.bfloat16

D = 1024
H = 16
HD = 64
NPAIR = 8  # head pairs
THETA = 10000.0
EPS = 1e-6

B_FULL, S_FULL = 4, 4096
N_CORES = 8
S_LOC_FULL = B_FULL * S_FULL // N_CORES  # 2048

REPLICA_GROUPS = [[0, 1], [2, 3], [4, 5], [6, 7]]

# pair p occupies cols [p*129, p*129+129) of the vk staging buffer; psum banks
# hold pairs (0,1,2), (3,4,5), (6,7)
VK_BANK_PAIRS = [(0, 3), (3, 6), (6, 8)]
VKW = 129  # 128 cols of vkT pair + 1 ksum col
NUMW = 65

RELU = mybir.ActivationFunctionType.Relu
COPY = mybir.ActivationFunctionType.Copy

STAGES = ["xt", "proj", "vk", "cc", "q", "num", "full"]


def build_program(s_loc=S_LOC_FULL, n_cores=N_CORES, stage="full", debug=False):
    """Build the SPMD Bass program for one core (all cores run the same code)."""
    LVL = STAGES.index(stage)

    def lvl(name):
        return STAGES.index(name) <= LVL

    T = s_loc // 128

    nc = bacc.Bacc("TRN2", target_bir_lowering=False, num_devices=n_cores)

    dbg = {}
    if debug:
        dbg["xt"] = nc.dram_tensor("xt_dbg", [128, 8 * s_loc], BF16, kind="ExternalOutput")
        dbg["kr"] = nc.dram_tensor("kr_dbg", [s_loc, D], BF16, kind="ExternalOutput")
        dbg["v"] = nc.dram_tensor("v_dbg", [s_loc, NPAIR * VKW], BF16, kind="ExternalOutput")
        dbg["vkacc"] = nc.dram_tensor("vkacc_dbg", [128, NPAIR * VKW], F32, kind="ExternalOutput")
        dbg["vkT"] = nc.dram_tensor("vkT_dbg", [128, H * NUMW], F32, kind="ExternalOutput")
        dbg["qT"] = nc.dram_tensor("qT_dbg", [128, T * D], BF16, kind="ExternalOutput")
        dbg["attn"] = nc.dram_tensor("attn_dbg", [s_loc, D], BF16, kind="ExternalOutput")

    x_d = nc.dram_tensor("x", [s_loc, D], BF16, kind="ExternalInput")
    wq_d = nc.dram_tensor("wq", [D, D], BF16, kind="ExternalInput")
    wk_d = nc.dram_tensor("wk", [D, D], BF16, kind="ExternalInput")
    wv_d = nc.dram_tensor("wv", [D, D], BF16, kind="ExternalInput")
    wo_d = nc.dram_tensor("wo", [D, D], BF16, kind="ExternalInput")
    # host pre-tiled: cos_t[p, t*32+j] = cos(t*128+p, j)
    cos_d = nc.dram_tensor("cos_t", [128, T * 32], F32, kind="ExternalInput")
    sin_d = nc.dram_tensor("sin_t", [128, T * 32], F32, kind="ExternalInput")
    y_d = nc.dram_tensor("y", [s_loc, D], F32, kind="ExternalOutput")

    def mm(dst, lhsT, rhs, start, stop):
        nc.tensor.matmul(dst, lhsT=lhsT, rhs=rhs, start=start, stop=stop)

    with tile.TileContext(nc) as tc:
        with (
            tc.tile_pool(name="const", bufs=1) as constp,
            tc.tile_pool(name="wpool", bufs=1) as wpool,
            tc.tile_pool(name="xtp", bufs=1) as xtp,
            tc.tile_pool(name="qtp", bufs=1) as qtp,
            tc.tile_pool(name="vkp", bufs=1) as vkp,
            tc.tile_pool(name="work", bufs=2) as wk,
            tc.tile_pool(name="io", bufs=3) as iop,
            tc.tile_pool(name="psP", bufs=3, space="PSUM") as psP,
            tc.tile_pool(name="dram", bufs=1, space="DRAM") as dramp,
        ):
            # rope tables (host pre-tiled, single contiguous DMA each)
            cos_all = constp.tile([128, T * 32], F32)
            nc.scalar.dma_start(cos_all[:], cos_d[:])
            sin_all = constp.tile([128, T * 32], F32)
            nc.scalar.dma_start(sin_all[:], sin_d[:])

            # xT resident, produced per tile by bf16 PE transposes (XBAR DMA
            # transposes corrupt each other across queues and cost ~1.25us of
            # queue time per call; PE does a tile in 0.54us)
            xt = xtp.tile([128, T * D], BF16, tag="xt")  # [p, (t, c*128+s)]

            ident = constp.tile([128, 128], F32)
            make_identity(nc, ident[:])
            ident_b = constp.tile([128, 128], BF16)
            nc.vector.tensor_copy(ident_b[:], ident[:])

            # weights: wk/wv needed immediately -- wk on the gpsimd queue, wv
            # on the sync queue (x tiles are small), both in quarters so tile
            # 0's matmuls start as soon as the first chunk lands
            def load_w(dram_t, tag, eng, parts):
                w_sb = wpool.tile([128, 8, D], BF16, tag=tag, name=tag)
                src = dram_t[:].rearrange("(c p) n -> p c n", p=128)
                step = 8 // parts
                for i in range(parts):
                    eng.dma_start(
                        w_sb[:, i * step : (i + 1) * step],
                        src[:, i * step : (i + 1) * step],
                    )
                return w_sb

            cc_in = dramp.tile([128, NPAIR * VKW], F32, tag="cci")
            cc_out = dramp.tile([128, NPAIR * VKW], F32, tag="cco")

            def rope_bank(psrc, t, dst_sb, nb):
                """rope one 512-col bank. Pool can't read PSUM, so the four
                psum-reading multiplies run on DVE (psum freed after them) and
                the SBUF-only sub/add run on Pool.

                psrc [128, 512] psum = 8 heads x (32 evens | 32 odds).
                dst_sb cols [nb*512, (nb+1)*512) get the roped result (bf16).
                """
                csb = cos_all[:, t * 32 : (t + 1) * 32]
                ssb = sin_all[:, t * 32 : (t + 1) * 32]
                e3 = psrc[:].rearrange("p (h d) -> p h d", h=8)[:, :, 0:32]
                o3 = psrc[:].rearrange("p (h d) -> p h d", h=8)[:, :, 32:64]
                cb = csb.unsqueeze(1).broadcast_to([128, 8, 32])
                sb_ = ssb.unsqueeze(1).broadcast_to([128, 8, 32])
                d3 = dst_sb[:, nb * 512 : (nb + 1) * 512].rearrange(
                    "p (h d) -> p h d", h=8
                )
                ts = [wk.tile([128, 256], F32, tag=f"rt{i}", name=f"rt{i}")
                      for i in range(4)]
                t13, t23, t33, t43 = (
                    t[:].rearrange("p (h d) -> p h d", h=8) for t in ts
                )
                nc.vector.tensor_mul(t13, e3, cb)
                nc.vector.tensor_mul(t23, o3, sb_)
                nc.vector.tensor_mul(t33, e3, sb_)
                nc.vector.tensor_mul(t43, o3, cb)
                nc.gpsimd.tensor_sub(d3[:, :, 0:32], t13, t23)
                nc.gpsimd.tensor_add(d3[:, :, 32:64], t33, t43)

            # ---------------- phase 1: k/v projections, vk accumulation -----
            # NOTE: opening a second accumulation group (start=True) in a PSUM
            # bank while another region's group is open DESTROYS the open
            # group's data -- so vk partials are single-shot groups per tile,
            # accumulated into SBUF on DVE.
            phase1 = tc.tile_pool(name="psVK", bufs=3, space="PSUM")
            psVK = phase1.__enter__()
            phase1x = tc.tile_pool(name="psX", bufs=2, space="PSUM")
            psX = phase1x.__enter__()
            vkacc = vkp.tile([128, NPAIR * VKW], F32, tag="vkacc")
            nc.vector.memset(vkacc[:], 0.0)
            prev = None  # (kr, v_sb) of previous tile, vk emitted one tile late

            def emit_vk(t, kr_sb, v_sb):
                for bi, (p0, p1) in enumerate(VK_BANK_PAIRS):
                    pvt = psVK.tile(
                        [128, (p1 - p0) * VKW], F32, tag="pvt", name=f"pvt{t}_{bi}"
                    )
                    for p in range(p0, p1):
                        mm(
                            pvt[:, (p - p0) * VKW : (p - p0 + 1) * VKW],
                            kr_sb[:, p * 128 : (p + 1) * 128],
                            v_sb[:, p * VKW : (p + 1) * VKW],
                            start=True,
                            stop=True,
                        )
                    nc.vector.tensor_add(
                        vkacc[:, p0 * VKW : p1 * VKW],
                        vkacc[:, p0 * VKW : p1 * VKW],
                        pvt[:],
                    )

            def fetch_x(t):
                xb = iop.tile([128, D], BF16, tag="xb", name=f"xb{t}")
                nc.sync.dma_start(xb[:], x_d[t * 128 : (t + 1) * 128, :])
                return xb

            def emit_xpose(t, xb):
                """PE-transpose x tile t into xt (psum->sbuf copy on scalar)."""
                for g in range(2):
                    pxt = psX.tile([128, 512], BF16, tag="px", name=f"px{t}_{g}")
                    for cc in range(4):
                        c = g * 4 + cc
                        nc.tensor.transpose(
                            pxt[:, cc * 128 : (cc + 1) * 128],
                            xb[:, c * 128 : (c + 1) * 128],
                            ident_b[:],
                        )
                    nc.scalar.activation(
                        xt[:, t * D + g * 512 : t * D + (g + 1) * 512],
                        pxt[:],
                        COPY,
                    )

            xbs = {}
            if lvl("proj") and T > 0:
                for tt in range(min(2, T)):
                    xbs[tt] = fetch_x(tt)

            wk_sb = load_w(wk_d, "wa", nc.gpsimd, 4)
            wv_sb = load_w(wv_d, "wb", nc.sync, 4)
            wq_sb = load_w(wq_d, "wc", nc.gpsimd, 2)
            wo_sb = load_w(wo_d, "wd", nc.gpsimd, 2)

            if lvl("proj") and T > 0:
                emit_xpose(0, xbs.pop(0))

            for t in range(T if lvl("proj") else 0):
                xt_c = lambda c: xt[:, t * D + c * 128 : t * D + (c + 1) * 128]

                # prefetch x two tiles ahead; transpose the NEXT tile first so
                # its psum->sbuf copy overlaps this tile's projection matmuls
                if t + 2 < T:
                    xbs[t + 2] = fetch_x(t + 2)
                if t + 1 < T:
                    emit_xpose(t + 1, xbs.pop(t + 1))

                # k projection + rope + relu
                kr_sb = wk.tile([128, D], BF16, tag="kr")
                for nb in range(2):
                    pk = psP.tile([128, 512], F32, tag="pp")
                    for c in range(8):
                        mm(
                            pk[:],
                            xt_c(c),
                            wk_sb[:, c, nb * 512 : (nb + 1) * 512],
                            start=(c == 0),
                            stop=(c == 7),
                        )
                    rope_bank(pk, t, kr_sb, nb)
                    nc.scalar.activation(
                        kr_sb[:, nb * 512 : (nb + 1) * 512],
                        kr_sb[:, nb * 512 : (nb + 1) * 512],
                        RELU,
                    )

                # v projection -> v_sb with ones cols at p*129+128
                v_sb = wk.tile([128, NPAIR * VKW], BF16, tag="v")
                for nb in range(2):
                    pv = psP.tile([128, 512], F32, tag="pp")
                    for c in range(8):
                        mm(
                            pv[:],
                            xt_c(c),
                            wv_sb[:, c, nb * 512 : (nb + 1) * 512],
                            start=(c == 0),
                            stop=(c == 7),
                        )
                    dst = v_sb[:, nb * 4 * VKW : (nb * 4 + 4) * VKW].rearrange(
                        "p (q c) -> p q c", q=4
                    )[:, :, 0:128]
                    nc.scalar.activation(
                        dst, pv[:].rearrange("p (q c) -> p q c", q=4), COPY
                    )
                nc.gpsimd.memset(
                    v_sb[:].rearrange("p (q c) -> p q c", q=8)[:, :, 128:129], 1.0
                )

                if debug:
                    nc.scalar.dma_start(
                        dbg["kr"][t * 128 : (t + 1) * 128, :], kr_sb[:]
                    )
                    nc.scalar.dma_start(
                        dbg["v"][t * 128 : (t + 1) * 128, :], v_sb[:]
                    )

                # vk partial products for the PREVIOUS tile (kr/v long ready,
                # so the PE never waits on rope/relu)
                if lvl("vk"):
                    if prev is not None:
                        emit_vk(prev[0], prev[1], prev[2])
                    prev = (t, kr_sb, v_sb)
            if lvl("vk") and prev is not None:
                emit_vk(prev[0], prev[1], prev[2])

            # ---------------- all-reduce vk over sequence-half pairs --------
            if lvl("cc"):
                if debug:
                    nc.scalar.dma_start(dbg["vkacc"][:], vkacc[:])
                nc.gpsimd.dma_start(cc_in[:], vkacc[:])
                nc.gpsimd.collective_compute(
                    "AllReduce",
                    mybir.AluOpType.add,
                    replica_groups=REPLICA_GROUPS,
                    ins=[cc_in.opt()],
                    outs=[cc_out.opt()],
                )
            phase1x.__exit__(None, None, None)
            phase1.__exit__(None, None, None)

            # ---------------- phase 2a: q proj + rope + qT ------------------
            # (independent of the collective -- this is the PE work reserve
            # that hides the all-reduce. Its psum comes from a pool that only
            # opens after phase 1's banks free, which stops the scheduler from
            # hoisting q matmuls into phase-1 stall windows and spending the
            # reserve early. relu is folded into the post-transpose copy.)
            phase2a = tc.tile_pool(name="psQ", bufs=2, space="PSUM")
            psQ = phase2a.__enter__()
            phase2q = tc.tile_pool(name="psQ2", bufs=3, space="PSUM")
            psQ2 = phase2q.__enter__()
            qT = qtp.tile([128, T * D], BF16, tag="qT")
            for t in range(T if lvl("q") else 0):
                qr_sb = wk.tile([128, D], BF16, tag="qr")
                for nb in range(2):
                    pq = psQ2.tile([128, 512], F32, tag="qq")
                    for c in range(8):
                        mm(
                            pq[:],
                            xt[:, t * D + c * 128 : t * D + (c + 1) * 128],
                            wq_sb[:, c, nb * 512 : (nb + 1) * 512],
                            start=(c == 0),
                            stop=(c == 7),
                        )
                    rope_bank(pq, t, qr_sb, nb)
                for g in range(2):
                    pqt = psQ.tile([128, 512], BF16, tag="pqt", name=f"pqt{t}_{g}")
                    for cc in range(4):
                        c = g * 4 + cc
                        nc.tensor.transpose(
                            pqt[:, cc * 128 : (cc + 1) * 128],
                            qr_sb[:, c * 128 : (c + 1) * 128],
                            ident_b[:],
                        )
                    nc.scalar.activation(
                        qT[:, t * D + g * 512 : t * D + (g + 1) * 512],
                        pqt[:],
                        RELU,
                    )

            # collective readback + vkT reorganization. high_priority so the
            # DVE/scalar pick these up the moment the collective lands instead
            # of draining their queued phase-2a work first.
            if lvl("cc"):
                with tc.high_priority():
                    vkred = vkp.tile([128, NPAIR * VKW], F32, tag="vkred")
                    nc.scalar.dma_start(vkred[:], cc_out[:])
                    # head h at partitions (h%2)*64, other half zeroed so num
                    # can contract K=128 (all matmuls at row base 0)
                    vkT_sb = vkp.tile([128, H * NUMW], BF16, tag="vkT")
                    nc.vector.memset(vkT_sb[:], 0.0)
                    for h in range(H):
                        p = h // 2
                        if h % 2 == 0:
                            nc.vector.tensor_copy(
                                vkT_sb[0:64, h * NUMW : h * NUMW + 64],
                                vkred[0:64, p * VKW : p * VKW + 64],
                            )
                            nc.vector.tensor_copy(
                                vkT_sb[0:64, h * NUMW + 64 : h * NUMW + 65],
                                vkred[0:64, p * VKW + 128 : p * VKW + 129],
                            )
                        else:
                            nc.vector.tensor_copy(
                                vkT_sb[64:128, h * NUMW : h * NUMW + 65],
                                vkred[64:128, p * VKW + 64 : p * VKW + 129],
                            )
                if debug:
                    vkT_f = vkp.tile([128, H * NUMW], F32, tag="vkTf")
                    nc.vector.tensor_copy(vkT_f[:], vkT_sb[:])
                    nc.scalar.dma_start(dbg["vkT"][:], vkT_f[:])
                    nc.scalar.dma_start(dbg["qT"][:], qT[:])
                    nc.scalar.dma_start(dbg["xt"][:], xt[:])

            # ---------------- phase 2b: num, attn, output -------------------
            # num psum is copied to SBUF immediately (DVE) so the whole attn
            # chain (den/rec/mul) runs on the otherwise-idle Pool engine from
            # SBUF, and attnT+out for tile t-1 run between num(t) and num(t+1)
            # -- two full pipeline periods of distance, so the PE never stalls
            # (any stall drops the PE to its mid p-state for ~3us).
            def emit_num(t, psN):
                num_sb = wk.tile([128, H * NUMW], F32, tag="numsb")
                for bi, (p0, p1) in enumerate(VK_BANK_PAIRS):
                    pn = psN.tile(
                        [128, (p1 - p0) * 2 * NUMW],
                        F32,
                        tag=f"num{bi}",
                        name=f"num{t}_{bi}",
                    )
                    for p in range(p0, p1):
                        mm(
                            pn[:, (p - p0) * 2 * NUMW : (p - p0 + 1) * 2 * NUMW],
                            qT[:, t * D + p * 128 : t * D + (p + 1) * 128],
                            vkT_sb[:, 2 * p * NUMW : 2 * (p + 1) * NUMW],
                            start=True,
                            stop=True,
                        )
                    nc.vector.tensor_copy(
                        num_sb[:, 2 * p0 * NUMW : 2 * p1 * NUMW], pn[:]
                    )
                # denominators -> reciprocal -> attn = num * rec (all Pool)
                den = wk.tile([128, H], F32, tag="den")
                nc.gpsimd.tensor_scalar_add(den[:], num_sb[:, 64 :: NUMW], EPS)
                rec = wk.tile([128, H], F32, tag="rec")
                nc.vector.reciprocal(rec[:], den[:])
                attn_sb = wk.tile([128, D], BF16, tag="attn")
                nc.gpsimd.tensor_mul(
                    attn_sb[:].rearrange("p (h e) -> p h e", e=64),
                    num_sb[:].rearrange("p (h e) -> p h e", e=NUMW)[:, :, 0:64],
                    rec[:].unsqueeze(2).broadcast_to([128, H, 64]),
                )
                if debug:
                    nc.scalar.dma_start(
                        dbg["attn"][t * 128 : (t + 1) * 128, :], attn_sb[:]
                    )
                return attn_sb

            def emit_out(t, attn_sb):
                attnT_sb = wk.tile([128, D], BF16, tag="attnT")
                for g in range(2):
                    pat = psQ.tile([128, 512], BF16, tag="pqt", name=f"pat{t}_{g}")
                    for cc in range(4):
                        c = g * 4 + cc
                        nc.tensor.transpose(
                            pat[:, cc * 128 : (cc + 1) * 128],
                            attn_sb[:, c * 128 : (c + 1) * 128],
                            ident_b[:],
                        )
                    nc.scalar.activation(
                        attnT_sb[:, g * 512 : (g + 1) * 512], pat[:], COPY
                    )
                out_sb = wk.tile([128, D], F32, tag="out")
                for nb in range(2):
                    po = psP.tile([128, 512], F32, tag="pp")
                    for c in range(8):
                        mm(
                            po[:],
                            attnT_sb[:, c * 128 : (c + 1) * 128],
                            wo_sb[:, c, nb * 512 : (nb + 1) * 512],
                            start=(c == 0),
                            stop=(c == 7),
                        )
                    nc.scalar.activation(
                        out_sb[:, nb * 512 : (nb + 1) * 512], po[:], COPY
                    )
                nc.scalar.dma_start(y_d[t * 128 : (t + 1) * 128, :], out_sb[:])

            if lvl("num"):
                phase2q.__exit__(None, None, None)
                with tc.tile_pool(name="psN", bufs=1, space="PSUM") as psN:
                    pending = None  # (t, attn_sb) emitted-num awaiting out
                    for t in range(T if lvl("full") else 1):
                        attn_sb = emit_num(t, psN)
                        if pending is not None:
                            emit_out(pending[0], pending[1])
                        pending = (t, attn_sb)
                    if pending is not None:
                        emit_out(pending[0], pending[1])
            else:
                phase2q.__exit__(None, None, None)
            phase2a.__exit__(None, None, None)

    nc.compile()
    return nc


# ---------------------------------------------------------------------------
# host side
# ---------------------------------------------------------------------------


def _head_perm():
    """De-interleave permutation for Wq/Wk columns (per head: evens then odds)."""
    perm = np.zeros(D, dtype=np.int64)
    for h in range(H):
        for j in range(32):
            perm[h * HD + j] = h * HD + 2 * j
            perm[h * HD + 32 + j] = h * HD + 2 * j + 1
    return perm


def _rope_tables(s_total):
    freqs = 1.0 / (THETA ** (np.arange(0, HD, 2, dtype=np.float64) / HD))
    ang = np.arange(s_total, dtype=np.float64)[:, None] * freqs[None, :]
    return (
        np.cos(ang).astype(np.float32),
        np.sin(ang).astype(np.float32),
    )


def _tile_table(tab, s_loc):
    # [s_loc, 32] -> [128, T*32] with tab_t[p, t*32+j] = tab[t*128+p, j]
    T = s_loc // 128
    return np.ascontiguousarray(
        tab.reshape(T, 128, 32).transpose(1, 0, 2).reshape(128, T * 32)
    )


def make_in_maps(x, Wq, Wk, Wv, Wo, n_cores=N_CORES):
    import ml_dtypes

    wdt = ml_dtypes.bfloat16
    x = np.asarray(x, np.float32)
    b, s, d = x.shape
    s_loc = b * s // n_cores
    halves = n_cores // b  # sequence splits per batch
    perm = _head_perm()
    wq_p = np.ascontiguousarray(np.asarray(Wq, np.float32)[:, perm]).astype(wdt)
    wk_p = np.ascontiguousarray(np.asarray(Wk, np.float32)[:, perm]).astype(wdt)
    Wv = np.ascontiguousarray(Wv).astype(wdt)
    Wo = np.ascontiguousarray(Wo).astype(wdt)
    cos_full, sin_full = _rope_tables(s)
    in_maps = []
    for c in range(n_cores):
        bi, hi = c // halves, c % halves
        r0 = hi * s_loc
        in_maps.append(
            {
                "x": np.ascontiguousarray(x[bi, r0 : r0 + s_loc]).astype(wdt),
                "wq": wq_p,
                "wk": wk_p,
                "wv": Wv,
                "wo": Wo,
                "cos_t": _tile_table(cos_full[r0 : r0 + s_loc], s_loc),
                "sin_t": _tile_table(sin_full[r0 : r0 + s_loc], s_loc),
            }
        )
    return in_maps, s_loc


_CACHED = {}


def kernel(x, Wq, Wk, Wv, Wo):
    from concourse.bass_utils import run_bass_kernel_spmd

    x = np.asarray(x, dtype=np.float32)
    in_maps, s_loc = make_in_maps(x, Wq, Wk, Wv, Wo)
    key = (s_loc, N_CORES)
    if key not in _CACHED:
        _CACHED[key] = build_program(s_loc=s_loc, n_cores=N_CORES)
    nc = _CACHED[key]
    res = run_bass_kernel_spmd(nc, in_maps, list(range(N_CORES)))
    b, s, d = x.shape
    halves = N_CORES // b
    out = np.empty((b, s, d), dtype=np.float32)
    for c in range(N_CORES):
        bi, hi = c // halves, c % halves
        out[bi, hi * s_loc : (hi + 1) * s_loc] = res.results[c]["y"]
    return out
